# revision 4
# baseline (speedup 1.0000x reference)
"""Trainium2 Bass kernel for CubeFaceNN.

Computes, for x of shape [8, 1, 128, 128, 128] (f32):
    out[b, i, p] = relu(x[b, 0, p] - x[b, 0, p + OFF[i]])   (zero padded)
with OFF = [(0,-1,-1), (-1,0,-1), (1,-1,-1), (-1,1,-1), (-1,-1,0), (-1,-1,1)]
(derived from the reference's adj % 3 - 1 indexing).

Sharding: pure data parallel — batch b -> NeuronCore b (8 cores).

Layout: depth d on the 128 SBUF partitions, (h, w) in the free dims. x is
resident in SBUF; the partition-shifted copy xp[d] = x[d+1] (serving all
five od != 0 channels via the substituted frame out[i, d'+1] =
relu(xp[d'] - x[d', h+oh, w+ow])) is built ON-CHIP by the idle PE array
with a one-subdiagonal shift matrix (exact: one-hot rows) instead of
re-reading 8 MiB from HBM. Output is computed in f32 and rounded once to
fp16 (per-element rel err <= 2^-11, gate is 2e-2), halving store traffic.

Probe-measured DMA facts on this silicon (all paths share 16 SDMA engines):
  - SWDGE loads ~14 GB/s/engine at any descriptor size; HWDGE loads
    ~21 GB/s; stores ~19.6 shallow / ~24.6 GB/s deep-queued at 8 KiB.
  - Both HWDGE rings (qSync via nc.sync, qScalar via nc.scalar) spread
    descriptors over all 16 engines and run concurrently, ~1 DMA in
    flight per ring -> big full-partition DMAs, alternate the rings.
  => ALL bulk DMA goes HWDGE (loads: 4x 2 MiB; stores: 24x 1 MiB units),
     which leaves the GpSimd Q7 cores with zero SWDGE descriptor work so
     they serve as a third compute engine.

Compute split (measured: DVE f32 ~119 Gel/s, fp16 2x; ACT ~145 Gel/s any
dtype; GpSimd ~153 Gel/s): subs ch0/2/4/5 on DVE, ch1/3 on GpSimd; relu
ch0 DVE, ch2 GpSimd, ch1/3/4/5 ACT; ACT also does the 32 PSUM->SBUF xp
copies and d-boundary planes. Channel work is cut into 24 h-quarter units
(8 och buffers) so sub -> relu -> store pipelines deeply.
"""

import numpy as np

import concourse.bacc as bacc
import concourse.mybir as mybir
import concourse.tile as tile
from concourse.bass_utils import run_bass_kernel_spmd

D = H = W = 128
HW = H * W
HALF = 64
UH = 32  # unit = h-quarter
UF = UH * W
NU = H // UH
N_CORES = 8
MMF = 512  # matmul moving free size (one PSUM bank of f32)
F32 = mybir.dt.float32
F16 = mybir.dt.float16

# (od, oh, ow) per output channel
OFFSETS = [(0, -1, -1), (-1, 0, -1), (1, -1, -1), (-1, 1, -1), (-1, -1, 0), (-1, -1, 1)]
GPSIMD_SUB = (1, 3)
RELU_ENG = {0: "vector", 2: "gpsimd"}  # default: scalar

_NC_CACHE = {}


def build_nc(debug=False):
    nc = bacc.Bacc("TRN2", target_bir_lowering=False, debug=debug)
    x = nc.dram_tensor("x", [D, H, W], F32, kind="ExternalInput")
    out = nc.dram_tensor("out", [6, D, H, W], F16, kind="ExternalOutput")
    # shift matrix: sh[k, m] = 1 iff k == m+1, so (sh.T @ x)[m] = x[m+1]
    sh_dram = nc.inline_tensor(np.eye(D, k=-1, dtype=np.float32), name="shift")

    sub = mybir.AluOpType.subtract
    relu = mybir.ActivationFunctionType.Relu
    rings = [nc.sync, nc.scalar]

    with tile.TileContext(nc) as tc:
        with (
            tc.tile_pool(name="xt", bufs=1) as xt_pool,
            tc.tile_pool(name="xp", bufs=1) as xp_pool,
            tc.tile_pool(name="sh", bufs=1) as sh_pool,
            tc.tile_pool(name="och", bufs=8) as och_pool,
            tc.tile_pool(name="pf32", bufs=2) as pf32_pool,
            tc.tile_pool(name="pf16", bufs=2) as pf16_pool,
            tc.tile_pool(name="ps", bufs=4, space="PSUM") as ps_pool,
        ):
            sht = sh_pool.tile([D, D], F32)
            nc.sync.dma_start(out=sht[:], in_=sh_dram[:])

            # x resident: 4x 2 MiB full-partition DMAs (16 KiB descriptors),
            # alternating the two HWDGE rings
            xt = xt_pool.tile([D, H, W], F32)
            for c in range(4):
                hsl = slice(c * 32, (c + 1) * 32)
                rings[c % 2].dma_start(out=xt[:, hsl], in_=x[:, hsl])
            xt2 = xt.rearrange("d h w -> d (h w)")

            # d-boundary planes: out[i, 0] = relu(x[0]) for od=-1 channels,
            # out[2, 127] = relu(x[127]); h on partitions.
            p0s = pf32_pool.tile([H, W], F32)
            p0 = pf16_pool.tile([H, W], F16)
            nc.sync.dma_start(out=p0s[:], in_=x[0])
            nc.scalar.activation(p0[:], p0s[:], relu)
            p1s = pf32_pool.tile([H, W], F32)
            p1 = pf16_pool.tile([H, W], F16)
            nc.sync.dma_start(out=p1s[:], in_=x[D - 1])
            nc.scalar.activation(p1[:], p1s[:], relu)
            for i, (od, _, _) in enumerate(OFFSETS):
                if od == -1:
                    nc.sync.dma_start(out=out[i, 0], in_=p0[:])
            nc.sync.dma_start(out=out[2, D - 1], in_=p1[:])

            # xp[d] = x[d+1] via PE shift matmul (f32, exact), PSUM -> SBUF on ACT
            xp = xp_pool.tile([D, H, W], F32)
            xp2 = xp.rearrange("d h w -> d (h w)")
            for c in range(HW // MMF):
                fsl = slice(c * MMF, (c + 1) * MMF)
                ps = ps_pool.tile([D, MMF], F32)
                nc.tensor.matmul(
                    out=ps[:], lhsT=sht[:], rhs=xt2[:, fsl], start=True, stop=True
                )
                nc.scalar.copy(out=xp2[:, fsl], in_=ps[:])

            unit_no = 0

            def emit_unit(i, u):
                nonlocal unit_no
                od, oh, ow = OFFSETS[i]
                dc = D if od == 0 else D - 1
                delta = oh * W + ow
                A3 = xp if od == -1 else xt  # aligned with the output frame
                S2 = xp2 if od == 1 else xt2  # d-shifted operand
                A2 = A3.rearrange("d h w -> d (h w)")

                hs, he = max(0, -oh), H - max(0, oh)
                f0, f1 = u * UF, (u + 1) * UF
                lo = max(f0, -delta)
                hi = min(f1, HW - delta)

                sub_eng = nc.gpsimd if i in GPSIMD_SUB else nc.vector
                relu_eng = getattr(nc, RELU_ENG.get(i, "scalar"))

                och = och_pool.tile([D, UH, W], F16)
                och2 = och.rearrange("d h w -> d (h w)")
                sub_eng.tensor_tensor(
                    out=och2[0:dc, lo - f0 : hi - f0],
                    in0=A2[0:dc, lo:hi],
                    in1=S2[0:dc, lo + delta : hi + delta],
                    op=sub,
                )

                # strips: shifted source is zero-padding there -> relu(A)
                def strip(osel, asel):
                    if relu_eng is nc.scalar:
                        nc.scalar.activation(och[osel], A3[asel], relu)
                    else:
                        relu_eng.tensor_scalar_max(och[osel], A3[asel], 0.0)

                r0 = u * UH
                if oh == -1 and u == 0:
                    strip((slice(0, dc), slice(0, 1)), (slice(0, dc), slice(0, 1)))
                if oh == 1 and u == NU - 1:
                    strip(
                        (slice(0, dc), slice(UH - 1, UH)),
                        (slice(0, dc), slice(H - 1, H)),
                    )
                if ow != 0:
                    wb = 0 if ow == -1 else W - 1
                    rs, re = max(hs, r0), min(he, r0 + UH)
                    strip(
                        (slice(0, dc), slice(rs - r0, re - r0), slice(wb, wb + 1)),
                        (slice(0, dc), slice(rs, re), slice(wb, wb + 1)),
                    )
                # interior relu (in place, fp16)
                osel = och2[0:dc, lo - f0 : hi - f0]
                if relu_eng is nc.scalar:
                    nc.scalar.activation(osel, osel, relu)
                else:
                    relu_eng.tensor_scalar_max(osel, osel, 0.0)

                # store: one full-partition 1 MiB DMA (8 KiB descriptors),
                # alternating rings
                d0 = 1 if od == -1 else 0
                rings[unit_no % 2].dma_start(
                    out=out[i, d0 : d0 + dc, r0 : r0 + UH], in_=och[0:dc]
                )
                unit_no += 1

            for u in range(NU):
                for i in range(6):
                    emit_unit(i, u)

    nc.compile()
    return nc


def _get_nc():
    if "nc" not in _NC_CACHE:
        _NC_CACHE["nc"] = build_nc()
    return _NC_CACHE["nc"]


def kernel(x: np.ndarray) -> np.ndarray:
    assert x.shape == (N_CORES, 1, D, H, W), x.shape
    nc = _get_nc()
    in_maps = [{"x": np.ascontiguousarray(x[b, 0], dtype=np.float32)} for b in range(N_CORES)]
    res = run_bass_kernel_spmd(nc, in_maps, core_ids=list(range(N_CORES)))
    return np.stack(
        [np.asarray(r["out"], dtype=np.float32) for r in res.results], axis=0
    )


# revision 8
# speedup vs baseline: 2.8913x; 2.8913x over previous
"""Trainium2 Bass kernel for CubeFaceNN.

Computes, for x of shape [8, 1, 128, 128, 128] (f32):
    out[b, i, p] = relu(x[b, 0, p] - x[b, 0, p + OFF[i]])   (zero padded)
with OFF = [(0,-1,-1), (-1,0,-1), (1,-1,-1), (-1,1,-1), (-1,-1,0), (-1,-1,1)]
(derived from the reference's adj % 3 - 1 indexing).

Sharding: pure data parallel — batch b -> NeuronCore b (8 cores).

Layout: depth d on the 128 SBUF partitions, (h, w) in the free dims. x is
resident in SBUF; the partition-shifted copy xp[d] = x[d+1] (serving all
five od != 0 channels via the substituted frame out[i, d'+1] =
relu(xp[d'] - x[d', h+oh, w+ow])) is built ON-CHIP by the idle PE array
with a one-subdiagonal shift matrix (exact: one-hot rows) instead of
re-reading 8 MiB from HBM. Output is computed in f32 and rounded once to
fp16 (per-element rel err <= 2^-11, gate is 2e-2), halving store traffic.

Probe-measured DMA facts on this silicon (all paths share 16 SDMA engines):
  - SWDGE stores ~19.6-24.6 GB/s/engine at 8 KiB descriptors when the
    queues stay deep -> all 24 unit stores go SWDGE (one full-partition
    1 MiB dma_start each, 8 och buffers keep several in flight).
  - SWDGE loads cap at ~14 GB/s/engine; HWDGE descriptors run ~21 but
    each HWDGE ring paces out at ~110-116 GB/s total. The two rings
    (nc.sync / nc.scalar) run concurrently -> x loads (4x 2 MiB,
    alternating rings) take ~36 us off the SWDGE path entirely, and the
    GpSimd Q7 cores keep only store-descriptor emission.
  - HBM/engine wall for the remaining traffic: ~100-107 us.

Compute split (measured: DVE f32 ~119 Gel/s, fp16 ~2x; ACT ~145 Gel/s any
dtype; GpSimd ~153 Gel/s): subs ch0/2/4/5 on DVE, ch1/3 on GpSimd
(interleaved with its store emissions); relu ch0/2 on DVE (fp16 2x),
ch1/3/4/5 on ACT; ACT also does the 32 PSUM->SBUF xp copies — interleaved
into the channel waves so wave-u relus only queue behind the copies they
need — and the d-boundary planes. Channel work runs in 4 waves of
h-quarter units; subs use flat contiguous APs with boundary strips
patched afterwards (relu(A) where the shifted source is zero padding).
"""

import numpy as np

import concourse.bacc as bacc
import concourse.mybir as mybir
import concourse.tile as tile
from concourse.bass_utils import run_bass_kernel_spmd

D = H = W = 128
HW = H * W
UH = 32  # unit = h-quarter
UF = UH * W
NU = H // UH
N_CORES = 8
MMF = 512  # matmul moving free size (one PSUM bank of f32)
NCHUNK = HW // MMF
F32 = mybir.dt.float32
F16 = mybir.dt.float16

# (od, oh, ow) per output channel
OFFSETS = [(0, -1, -1), (-1, 0, -1), (1, -1, -1), (-1, 1, -1), (-1, -1, 0), (-1, -1, 1)]
GPSIMD_SUB = (1, 3)
DVE_RELU = (0, 2)

_NC_CACHE = {}


def build_nc(debug=False):
    nc = bacc.Bacc("TRN2", target_bir_lowering=False, debug=debug)
    x = nc.dram_tensor("x", [D, H, W], F32, kind="ExternalInput")
    out = nc.dram_tensor("out", [6, D, H, W], F16, kind="ExternalOutput")
    # shift matrix: sh[k, m] = 1 iff k == m+1, so (sh.T @ x)[m] = x[m+1]
    sh_dram = nc.inline_tensor(np.eye(D, k=-1, dtype=np.float32), name="shift")

    sub = mybir.AluOpType.subtract
    relu = mybir.ActivationFunctionType.Relu
    rings = [nc.sync, nc.scalar]

    with tile.TileContext(nc) as tc:
        with (
            tc.tile_pool(name="xt", bufs=1) as xt_pool,
            tc.tile_pool(name="xp", bufs=1) as xp_pool,
            tc.tile_pool(name="sh", bufs=1) as sh_pool,
            tc.tile_pool(name="och", bufs=8) as och_pool,
            tc.tile_pool(name="pf32", bufs=2) as pf32_pool,
            tc.tile_pool(name="pf16", bufs=2) as pf16_pool,
            tc.tile_pool(name="ps", bufs=8, space="PSUM") as ps_pool,
        ):
            sht = sh_pool.tile([D, D], F32)
            nc.sync.dma_start(out=sht[:], in_=sh_dram[:])

            # x resident: 4x 2 MiB full-partition HWDGE DMAs (16 KiB
            # descriptors), alternating the two rings
            xt = xt_pool.tile([D, H, W], F32)
            for c in range(4):
                hsl = slice(c * 32, (c + 1) * 32)
                rings[c % 2].dma_start(out=xt[:, hsl], in_=x[:, hsl])
            xt2 = xt.rearrange("d h w -> d (h w)")

            # d-boundary planes: out[i, 0] = relu(x[0]) for od=-1 channels,
            # out[2, 127] = relu(x[127]); h on partitions, HWDGE rings.
            p0s = pf32_pool.tile([H, W], F32)
            p0 = pf16_pool.tile([H, W], F16)
            nc.sync.dma_start(out=p0s[:], in_=x[0])
            nc.scalar.activation(p0[:], p0s[:], relu)
            p1s = pf32_pool.tile([H, W], F32)
            p1 = pf16_pool.tile([H, W], F16)
            nc.scalar.dma_start(out=p1s[:], in_=x[D - 1])
            nc.scalar.activation(p1[:], p1s[:], relu)
            for i, (od, _, _) in enumerate(OFFSETS):
                if od == -1:
                    rings[i % 2].dma_start(out=out[i, 0], in_=p0[:])
            nc.scalar.dma_start(out=out[2, D - 1], in_=p1[:])

            # xp[d] = x[d+1] via PE shift matmul (f32, exact). All matmuls
            # emitted up front (PE free-runs); the PSUM->SBUF copies (ACT)
            # are interleaved into the waves below.
            xp = xp_pool.tile([D, H, W], F32)
            xp2 = xp.rearrange("d h w -> d (h w)")
            ps_tiles = []
            for c in range(NCHUNK):
                ps = ps_pool.tile([D, MMF], F32)
                nc.tensor.matmul(
                    out=ps[:],
                    lhsT=sht[:],
                    rhs=xt2[:, c * MMF : (c + 1) * MMF],
                    start=True,
                    stop=True,
                )
                ps_tiles.append(ps)

            def emit_copy(c):
                nc.scalar.copy(
                    out=xp2[:, c * MMF : (c + 1) * MMF], in_=ps_tiles[c][:]
                )

            def emit_compute(i, u, och):
                od, oh, ow = OFFSETS[i]
                dc = D if od == 0 else D - 1
                delta = oh * W + ow
                A3 = xp if od == -1 else xt  # aligned with the output frame
                S2 = xp2 if od == 1 else xt2  # d-shifted operand
                A2 = A3.rearrange("d h w -> d (h w)")

                hs, he = max(0, -oh), H - max(0, oh)
                f0, f1 = u * UF, (u + 1) * UF
                lo = max(f0, -delta)
                hi = min(f1, HW - delta)

                sub_eng = nc.gpsimd if i in GPSIMD_SUB else nc.vector
                on_dve = i in DVE_RELU

                och2 = och.rearrange("d h w -> d (h w)")
                sub_eng.tensor_tensor(
                    out=och2[0:dc, lo - f0 : hi - f0],
                    in0=A2[0:dc, lo:hi],
                    in1=S2[0:dc, lo + delta : hi + delta],
                    op=sub,
                )

                # strips: shifted source is zero-padding there -> relu(A)
                def strip(osel, asel):
                    if on_dve:
                        nc.vector.tensor_scalar_max(och[osel], A3[asel], 0.0)
                    else:
                        nc.scalar.activation(och[osel], A3[asel], relu)

                r0 = u * UH
                if oh == -1 and u == 0:
                    strip((slice(0, dc), slice(0, 1)), (slice(0, dc), slice(0, 1)))
                if oh == 1 and u == NU - 1:
                    strip(
                        (slice(0, dc), slice(UH - 1, UH)),
                        (slice(0, dc), slice(H - 1, H)),
                    )
                if ow != 0:
                    wb = 0 if ow == -1 else W - 1
                    rs, re = max(hs, r0), min(he, r0 + UH)
                    strip(
                        (slice(0, dc), slice(rs - r0, re - r0), slice(wb, wb + 1)),
                        (slice(0, dc), slice(rs, re), slice(wb, wb + 1)),
                    )
                # interior relu (in place, fp16)
                osel = och2[0:dc, lo - f0 : hi - f0]
                if on_dve:
                    nc.vector.tensor_scalar_max(osel, osel, 0.0)
                else:
                    nc.scalar.activation(osel, osel, relu)

            def emit_store(i, u, och):
                # one full-partition 1 MiB SWDGE DMA (8 KiB descriptors)
                od = OFFSETS[i][0]
                dc = D if od == 0 else D - 1
                d0 = 1 if od == -1 else 0
                r0 = u * UH
                nc.gpsimd.dma_start(
                    out=out[i, d0 : d0 + dc, r0 : r0 + UH], in_=och[0:dc]
                )

            # Waves of h-quarters. ACT program order: the 9 copies wave 0
            # reads, then per wave the relus interleaved with the 8 copies
            # the NEXT wave needs. gpsimd: its two subs first, stores at
            # wave end (ordered by expected relu completion).
            copy_next = 0

            def emit_copies(n):
                nonlocal copy_next
                for c in range(copy_next, min(NCHUNK, copy_next + n)):
                    emit_copy(c)
                copy_next = min(NCHUNK, copy_next + n)

            emit_copies(9)
            SUB_ORDER = (1, 3, 0, 2, 4, 5)  # gpsimd subs first
            STORE_ORDER = (0, 2, 1, 3, 4, 5)  # DVE-relu'd units first
            PER_UNIT_COPIES = (2, 1, 1, 2, 1, 1)
            for u in range(NU):
                tiles = {}
                for j, i in enumerate(SUB_ORDER):
                    tiles[i] = och_pool.tile([D, UH, W], F16, name="och")
                    emit_compute(i, u, tiles[i])
                    emit_copies(PER_UNIT_COPIES[j])
                for i in STORE_ORDER:
                    emit_store(i, u, tiles[i])

    nc.compile()
    return nc


def _get_nc():
    if "nc" not in _NC_CACHE:
        _NC_CACHE["nc"] = build_nc()
    return _NC_CACHE["nc"]


def kernel(x: np.ndarray) -> np.ndarray:
    assert x.shape == (N_CORES, 1, D, H, W), x.shape
    nc = _get_nc()
    in_maps = [{"x": np.ascontiguousarray(x[b, 0], dtype=np.float32)} for b in range(N_CORES)]
    res = run_bass_kernel_spmd(nc, in_maps, core_ids=list(range(N_CORES)))
    return np.stack(
        [np.asarray(r["out"], dtype=np.float32) for r in res.results], axis=0
    )


# revision 10
# speedup vs baseline: 3.6480x; 1.2617x over previous
"""Trainium2 Bass kernel for CubeFaceNN.

Computes, for x of shape [8, 1, 128, 128, 128] (f32):
    out[b, i, p] = relu(x[b, 0, p] - x[b, 0, p + OFF[i]])   (zero padded)
with OFF = [(0,-1,-1), (-1,0,-1), (1,-1,-1), (-1,1,-1), (-1,-1,0), (-1,-1,1)]
(derived from the reference's adj % 3 - 1 indexing).

Sharding: pure data parallel — batch b -> NeuronCore b (8 cores).

Layout: depth d on the 128 SBUF partitions, (h, w) in the free dims. x is
resident in SBUF; the partition-shifted copy xp[d] = x[d+1] (serving all
five od != 0 channels via the substituted frame out[i, d'+1] =
relu(xp[d'] - x[d', h+oh, w+ow])) is built ON-CHIP by the idle PE array
with a one-subdiagonal shift matrix (exact: one-hot rows) instead of
re-reading 8 MiB from HBM. Output is computed in f32 and rounded once to
fp16 (per-element rel err <= 2^-11, gate is 2e-2), halving store traffic.

Probe-measured DMA facts on this silicon (all paths share 16 SDMA engines):
  - SWDGE stores ~19.6-24.6 GB/s/engine at 8 KiB descriptors when the
    queues stay deep -> all 24 unit stores go SWDGE (one full-partition
    1 MiB dma_start each, 8 och buffers keep several in flight).
  - SWDGE loads cap at ~14 GB/s/engine; HWDGE descriptors run ~21 but
    each HWDGE ring paces out at ~110-116 GB/s total. The two rings
    (nc.sync / nc.scalar) run concurrently -> x loads (4x 2 MiB,
    alternating rings) take ~36 us off the SWDGE path entirely, and the
    GpSimd Q7 cores keep only store-descriptor emission.
  - HBM/engine wall for the remaining traffic: ~100-107 us.

Compute split (measured: DVE f32 ~119 Gel/s, fp16 ~2x; ACT ~145 Gel/s any
dtype; GpSimd ~153 Gel/s): subs ch0/2/4/5 on DVE, ch1/3 on GpSimd
(interleaved with its store emissions); relu ch0/2 on DVE (fp16 2x),
ch1/3/4/5 on ACT; ACT also does the 32 PSUM->SBUF xp copies — interleaved
into the channel waves so wave-u relus only queue behind the copies they
need — and the d-boundary planes. Channel work runs in 4 waves of
h-quarter units; subs use flat contiguous APs with boundary strips
patched afterwards (relu(A) where the shifted source is zero padding).
"""

import numpy as np

import concourse.bacc as bacc
import concourse.mybir as mybir
import concourse.tile as tile
from concourse.bass_utils import run_bass_kernel_spmd

D = H = W = 128
HW = H * W
UH = 32  # unit = h-quarter
UF = UH * W
NU = H // UH
N_CORES = 8
MMF = 512  # matmul moving free size (one PSUM bank of f32)
NCHUNK = HW // MMF
F32 = mybir.dt.float32
F16 = mybir.dt.float16

# (od, oh, ow) per output channel
OFFSETS = [(0, -1, -1), (-1, 0, -1), (1, -1, -1), (-1, 1, -1), (-1, -1, 0), (-1, -1, 1)]
GPSIMD_SUB = ()  # gpsimd tensor ops measured ~31 Gel/s (5x below spec) and
# block the store emissions queued behind them — keep it DMA-only
DVE_RELU = (0,)

_NC_CACHE = {}


def build_nc(debug=False):
    nc = bacc.Bacc("TRN2", target_bir_lowering=False, debug=debug)
    x = nc.dram_tensor("x", [D, H, W], F32, kind="ExternalInput")
    out = nc.dram_tensor("out", [6, D, H, W], F16, kind="ExternalOutput")
    # shift matrix: sh[k, m] = 1 iff k == m+1, so (sh.T @ x)[m] = x[m+1]
    sh_dram = nc.inline_tensor(np.eye(D, k=-1, dtype=np.float32), name="shift")

    sub = mybir.AluOpType.subtract
    relu = mybir.ActivationFunctionType.Relu
    rings = [nc.sync, nc.scalar]

    with tile.TileContext(nc) as tc:
        with (
            tc.tile_pool(name="xt", bufs=1) as xt_pool,
            tc.tile_pool(name="xp", bufs=1) as xp_pool,
            tc.tile_pool(name="sh", bufs=1) as sh_pool,
            tc.tile_pool(name="och", bufs=8) as och_pool,
            tc.tile_pool(name="pf32", bufs=2) as pf32_pool,
            tc.tile_pool(name="pf16", bufs=2) as pf16_pool,
            tc.tile_pool(name="ps", bufs=8, space="PSUM") as ps_pool,
        ):
            sht = sh_pool.tile([D, D], F32)
            nc.sync.dma_start(out=sht[:], in_=sh_dram[:])

            # x resident: 4x 2 MiB full-partition HWDGE DMAs (16 KiB
            # descriptors), alternating the two rings
            xt = xt_pool.tile([D, H, W], F32)
            for c in range(4):
                hsl = slice(c * 32, (c + 1) * 32)
                rings[c % 2].dma_start(out=xt[:, hsl], in_=x[:, hsl])
            xt2 = xt.rearrange("d h w -> d (h w)")

            # d-boundary planes: out[i, 0] = relu(x[0]) for od=-1 channels,
            # out[2, 127] = relu(x[127]); h on partitions, HWDGE rings.
            p0s = pf32_pool.tile([H, W], F32)
            p0 = pf16_pool.tile([H, W], F16)
            nc.sync.dma_start(out=p0s[:], in_=x[0])
            nc.scalar.activation(p0[:], p0s[:], relu)
            p1s = pf32_pool.tile([H, W], F32)
            p1 = pf16_pool.tile([H, W], F16)
            nc.scalar.dma_start(out=p1s[:], in_=x[D - 1])
            nc.scalar.activation(p1[:], p1s[:], relu)
            for i, (od, _, _) in enumerate(OFFSETS):
                if od == -1:
                    rings[i % 2].dma_start(out=out[i, 0], in_=p0[:])
            nc.scalar.dma_start(out=out[2, D - 1], in_=p1[:])

            # xp[d] = x[d+1] via PE shift matmul (f32, exact). All matmuls
            # emitted up front (PE free-runs); the PSUM->SBUF copies (ACT)
            # are interleaved into the waves below.
            xp = xp_pool.tile([D, H, W], F32)
            xp2 = xp.rearrange("d h w -> d (h w)")
            ps_tiles = []
            for c in range(NCHUNK):
                ps = ps_pool.tile([D, MMF], F32)
                nc.tensor.matmul(
                    out=ps[:],
                    lhsT=sht[:],
                    rhs=xt2[:, c * MMF : (c + 1) * MMF],
                    start=True,
                    stop=True,
                )
                ps_tiles.append(ps)

            def emit_copy(c):
                nc.scalar.copy(
                    out=xp2[:, c * MMF : (c + 1) * MMF], in_=ps_tiles[c][:]
                )

            def emit_compute(i, u, och):
                od, oh, ow = OFFSETS[i]
                dc = D if od == 0 else D - 1
                delta = oh * W + ow
                A3 = xp if od == -1 else xt  # aligned with the output frame
                S2 = xp2 if od == 1 else xt2  # d-shifted operand
                A2 = A3.rearrange("d h w -> d (h w)")

                hs, he = max(0, -oh), H - max(0, oh)
                f0, f1 = u * UF, (u + 1) * UF
                lo = max(f0, -delta)
                hi = min(f1, HW - delta)

                sub_eng = nc.gpsimd if i in GPSIMD_SUB else nc.vector
                on_dve = i in DVE_RELU

                och2 = och.rearrange("d h w -> d (h w)")
                sub_eng.tensor_tensor(
                    out=och2[0:dc, lo - f0 : hi - f0],
                    in0=A2[0:dc, lo:hi],
                    in1=S2[0:dc, lo + delta : hi + delta],
                    op=sub,
                )

                # strips: shifted source is zero-padding there -> relu(A)
                def strip(osel, asel):
                    if on_dve:
                        nc.vector.tensor_scalar_max(och[osel], A3[asel], 0.0)
                    else:
                        nc.scalar.activation(och[osel], A3[asel], relu)

                r0 = u * UH
                if oh == -1 and u == 0:
                    strip((slice(0, dc), slice(0, 1)), (slice(0, dc), slice(0, 1)))
                if oh == 1 and u == NU - 1:
                    strip(
                        (slice(0, dc), slice(UH - 1, UH)),
                        (slice(0, dc), slice(H - 1, H)),
                    )
                if ow != 0:
                    wb = 0 if ow == -1 else W - 1
                    rs, re = max(hs, r0), min(he, r0 + UH)
                    strip(
                        (slice(0, dc), slice(rs - r0, re - r0), slice(wb, wb + 1)),
                        (slice(0, dc), slice(rs, re), slice(wb, wb + 1)),
                    )
                # interior relu (in place, fp16)
                osel = och2[0:dc, lo - f0 : hi - f0]
                if on_dve:
                    nc.vector.tensor_scalar_max(osel, osel, 0.0)
                else:
                    nc.scalar.activation(osel, osel, relu)

            def emit_store(i, u, och):
                # one full-partition 1 MiB SWDGE DMA (8 KiB descriptors)
                od = OFFSETS[i][0]
                dc = D if od == 0 else D - 1
                d0 = 1 if od == -1 else 0
                r0 = u * UH
                nc.gpsimd.dma_start(
                    out=out[i, d0 : d0 + dc, r0 : r0 + UH], in_=och[0:dc]
                )

            # Waves of h-quarters. ACT program order: the 9 copies wave 0
            # reads, then per wave the relus interleaved with the 8 copies
            # the NEXT wave needs. gpsimd: its two subs first, stores at
            # wave end (ordered by expected relu completion).
            copy_next = 0

            def emit_copies(n):
                nonlocal copy_next
                for c in range(copy_next, min(NCHUNK, copy_next + n)):
                    emit_copy(c)
                copy_next = min(NCHUNK, copy_next + n)

            emit_copies(9)
            SUB_ORDER = (0, 2, 1, 3, 4, 5)  # xp-free channel first
            STORE_ORDER = (0, 2, 1, 3, 4, 5)  # DVE-relu'd unit first
            PER_UNIT_COPIES = (2, 1, 1, 2, 1, 1)
            for u in range(NU):
                tiles = {}
                for j, i in enumerate(SUB_ORDER):
                    tiles[i] = och_pool.tile([D, UH, W], F16, name="och")
                    emit_compute(i, u, tiles[i])
                    emit_copies(PER_UNIT_COPIES[j])
                for i in STORE_ORDER:
                    emit_store(i, u, tiles[i])

    nc.compile()
    return nc


def _get_nc():
    if "nc" not in _NC_CACHE:
        _NC_CACHE["nc"] = build_nc()
    return _NC_CACHE["nc"]


def kernel(x: np.ndarray) -> np.ndarray:
    assert x.shape == (N_CORES, 1, D, H, W), x.shape
    nc = _get_nc()
    in_maps = [{"x": np.ascontiguousarray(x[b, 0], dtype=np.float32)} for b in range(N_CORES)]
    res = run_bass_kernel_spmd(nc, in_maps, core_ids=list(range(N_CORES)))
    return np.stack(
        [np.asarray(r["out"], dtype=np.float32) for r in res.results], axis=0
    )
